# revision 4
# baseline (speedup 1.0000x reference)
"""EoMT criterion (Mask2Former-style loss) on 8 Trainium2 NeuronCores.

Math reduction: for each image with mask logits x [N=100, HW] and binary gt
masks y [M=20, HW], every term of the loss only needs
    A  = x @ y.T                  (since log p - log(1-p) = x)
    B  = sigmoid(x) @ y.T
    sp = sum_px softplus(x)  per row      (= -rowsum log(1-p))
    ps = sum_px sigmoid(x)   per row
    ys = sum_px y            per row (host, exact ints)
because
    bce_cost[n,m]  = (sp[n] - A[n,m]) / HW
    dice_cost[n,m] = 1 - (2 B[n,m] + 1) / (ps[n] + ys[m] + 1)
    matched-pair bce  = sum_k (sp[pi_k] - A[pi_k, gi_k]) / (K*HW)
    matched-pair dice from B/ps/ys at the matched indices.
The device reduces 250MB of inputs to one [21, 300] f32 tile per core; the
Hungarian assignment and the tiny class-logit terms run on host.

Sharding: 8 cores = 2 images x 4 HW-quarters. Host pre-transposes x to
pixel-major [HW, N] bf16 so the pixel (contraction) axis lands on SBUF
partitions, and appends a ones column to y ([HW, 21]) so row 20 of each
matmul output carries the per-row pixel sums (ps, sp).
"""

import numpy as np
import ml_dtypes

BF16 = ml_dtypes.bfloat16

N_CORES = 8
BS = 2
NQ = 100          # number of mask queries
NM = 20           # number of gt masks
NC1 = 7           # classes + no-object
H = W = 512
HW = H * W                      # 262144
PX_PER_CORE = HW // 4           # 65536
N_CHUNKS = 8
PX_PER_CHUNK = PX_PER_CORE // N_CHUNKS   # 8192
F = PX_PER_CHUNK // 128                  # 64 pixel-groups per chunk
NGROUP = PX_PER_CORE // 128              # 512 groups per core

CLS_W, MASK_W, DICE_W, NO_OBJ_W = 2.0, 5.0, 5.0, 0.1
NO_OBJ = 6

_NC_CACHE = {}


def _build_device_kernel(reps=1):
    """One SPMD program per core: inputs xt [65536,100] bf16 (pixel-major
    mask logits), yt [65536,21] bf16 (gt masks + ones col); output
    [21, 300] f32 = [A.T | Q.T | T.T] blocks where A = x @ y1.T,
    Q = sigmoid(-x) @ y1.T, T = ln(sigmoid(-x)) @ y1.T (row 20 = colsums).

    The compiler's activation tables have no softplus, so softplus/sigmoid
    come from q = sigmoid(-x): sigmoid(x) = 1-q, softplus(x) = -ln(q)."""
    import concourse.bacc as bacc
    import concourse.mybir as mybir
    import concourse.tile as tile

    nc = bacc.Bacc("TRN2", target_bir_lowering=False, debug=False,
                   num_devices=N_CORES)
    x_d = nc.dram_tensor("xt", (PX_PER_CORE, NQ), mybir.dt.bfloat16,
                         kind="ExternalInput")
    y_d = nc.dram_tensor("yt", (PX_PER_CORE, NM + 1), mybir.dt.bfloat16,
                         kind="ExternalInput")
    out_d = nc.dram_tensor("out_res", (NM + 1, 3 * NQ), mybir.dt.float32,
                           kind="ExternalOutput")

    AF = mybir.ActivationFunctionType
    bf16 = mybir.dt.bfloat16
    f32 = mybir.dt.float32

    with tile.TileContext(nc) as tc:
        with (
            tc.tile_pool(name="xpool", bufs=3) as xpool,
            tc.tile_pool(name="qpool", bufs=1) as qpool,
            tc.tile_pool(name="ypool", bufs=1) as ypool,
            tc.tile_pool(name="tpool", bufs=2) as tpool,
            tc.tile_pool(name="opool", bufs=1) as opool,
            tc.tile_pool(name="pspool", bufs=1, space="PSUM") as pspool,
        ):
            y_tile = ypool.tile([128, N_CHUNKS, F, NM + 1], bf16)
            nc.sync.dma_start(
                y_tile[:],
                y_d.ap().rearrange("(c p f) j -> p c f j",
                                   c=N_CHUNKS, p=128, f=F))

            psA = pspool.tile([NM + 1, NQ], f32)
            psQ = pspool.tile([NM + 1, NQ], f32)
            psT = pspool.tile([NM + 1, NQ], f32)

            x_view = x_d.ap().rearrange("(c p f) j -> c p f j",
                                        c=N_CHUNKS, p=128, f=F)
            q_tiles = [qpool.tile([128, F, NQ], bf16, name=f"q_{c}")
                       for c in range(N_CHUNKS)]

            for r in range(reps):
                first = r == 0
                last = r == reps - 1
                # phase 1: q = sigmoid(-x); accumulate A (raw x) and Q (q)
                for c in range(N_CHUNKS):
                    x_t = xpool.tile([128, F, NQ], bf16, name="x_t", tag="x")
                    nc.sync.dma_start(x_t[:], x_view[c])
                    q_t = q_tiles[c]
                    nc.scalar.activation(q_t[:], x_t[:], AF.Sigmoid,
                                         scale=-1.0)
                    for f in range(F):
                        g = c * F + f
                        st = first and g == 0
                        sp = last and g == NGROUP - 1
                        nc.tensor.matmul(psA[:], y_tile[:, c, f, :],
                                         x_t[:, f, :], start=st, stop=sp)
                        nc.tensor.matmul(psQ[:], y_tile[:, c, f, :],
                                         q_t[:, f, :], start=st, stop=sp)
                # phase 2: t = ln(q) = -softplus(x); accumulate T
                for c in range(N_CHUNKS):
                    q_t = q_tiles[c]
                    t_t = tpool.tile([128, F, NQ], bf16, name="t_t", tag="t")
                    nc.scalar.activation(t_t[:], q_t[:], AF.Ln)
                    for f in range(F):
                        g = c * F + f
                        st = first and g == 0
                        sp = last and g == NGROUP - 1
                        nc.tensor.matmul(psT[:], y_tile[:, c, f, :],
                                         t_t[:, f, :], start=st, stop=sp)

            out_sb = opool.tile([NM + 1, 3 * NQ], f32)
            nc.vector.tensor_copy(out_sb[:, 0:NQ], psA[:])
            nc.vector.tensor_copy(out_sb[:, NQ:2 * NQ], psQ[:])
            nc.vector.tensor_copy(out_sb[:, 2 * NQ:3 * NQ], psT[:])
            nc.sync.dma_start(out_d.ap(), out_sb[:])

    nc.compile()
    return nc


def _get_nc(reps=1):
    if reps not in _NC_CACHE:
        _NC_CACHE[reps] = _build_device_kernel(reps)
    return _NC_CACHE[reps]


def _prepare_in_maps(mask_logits, gt_masks):
    """Host-side marshalling: transpose to pixel-major, cast bf16, shard."""
    m2 = mask_logits.reshape(BS, NQ, HW)
    g2 = gt_masks.reshape(BS, NM, HW)
    in_maps = []
    for b in range(BS):
        for q in range(4):
            sl = slice(q * PX_PER_CORE, (q + 1) * PX_PER_CORE)
            xt = np.ascontiguousarray(m2[b, :, sl].T).astype(BF16)
            yt = np.empty((PX_PER_CORE, NM + 1), dtype=BF16)
            yt[:, :NM] = g2[b, :, sl].T
            yt[:, NM] = BF16(1.0)
            in_maps.append({"xt": xt, "yt": yt})
    return in_maps


def _run_device(in_maps, reps=1, trace=False):
    from concourse import bass_utils
    nc = _get_nc(reps)
    res = bass_utils.run_bass_kernel_spmd(
        nc, in_maps, core_ids=list(range(N_CORES)), trace=trace)
    return res


def _hungarian(cost):
    """Jonker-Volgenant shortest augmenting path; equivalent to scipy's
    linear_sum_assignment. cost [n, m] -> (row_ind, col_ind) sorted by row."""
    cost = np.asarray(cost, dtype=np.float64)
    transposed = cost.shape[0] > cost.shape[1]
    if transposed:
        cost = cost.T
    n, m = cost.shape
    INF = 1e18
    u = np.zeros(n + 1)
    v = np.zeros(m + 1)
    p = np.zeros(m + 1, dtype=np.int64)
    way = np.zeros(m + 1, dtype=np.int64)
    for i in range(1, n + 1):
        p[0] = i
        j0 = 0
        minv = np.full(m + 1, INF)
        used = np.zeros(m + 1, dtype=bool)
        while True:
            used[j0] = True
            i0 = p[j0]
            cand = cost[i0 - 1] - u[i0] - v[1:]
            upd = (~used[1:]) & (cand < minv[1:])
            minv[1:] = np.where(upd, cand, minv[1:])
            way[1:] = np.where(upd, j0, way[1:])
            masked = np.where(used[1:], INF, minv[1:])
            j1 = int(np.argmin(masked)) + 1
            delta = masked[j1 - 1]
            u[p[used]] += delta
            v[used] -= delta
            minv[1:][~used[1:]] -= delta
            j0 = j1
            if p[j0] == 0:
                break
        while j0:
            j1 = way[j0]
            p[j0] = p[j1]
            j0 = j1
    rows, cols = [], []
    for j in range(1, m + 1):
        if p[j] != 0:
            rows.append(p[j] - 1)
            cols.append(j - 1)
    rows = np.asarray(rows, dtype=np.int64)
    cols = np.asarray(cols, dtype=np.int64)
    if transposed:
        rows, cols = cols, rows
    order = np.argsort(rows)
    return rows[order], cols[order]


def _finish_on_host(core_outs, class_logits, gt_classes, gt_masks):
    """Combine per-core [21, 300] partials, assemble costs, match, and
    compute the four loss scalars."""
    g2 = gt_masks.reshape(BS, NM, HW)
    cls64 = class_logits.astype(np.float64)

    tc = tm = td = 0.0
    for b in range(BS):
        tot = np.zeros((NM + 1, 3 * NQ), dtype=np.float64)
        for q in range(4):
            tot += core_outs[4 * b + q].astype(np.float64)
        ys = g2[b].sum(axis=1).astype(np.float64)   # [M]
        A = tot[:NM, 0:NQ].T                 # [N, M] = x @ y.T
        Q = tot[:NM, NQ:2 * NQ].T            # [N, M] = sigmoid(-x) @ y.T
        qs = tot[NM, NQ:2 * NQ]              # [N]    = rowsum sigmoid(-x)
        B = ys[None, :] - Q                  # [N, M] = sigmoid(x) @ y.T
        ps = HW - qs                         # [N]    = rowsum sigmoid(x)
        sp = -tot[NM, 2 * NQ:3 * NQ]         # [N]    = rowsum softplus(x)

        # cost matrix
        cl = cls64[b]                        # [N, 7]
        z = cl - cl.max(axis=1, keepdims=True)
        ez = np.exp(z)
        prob = ez / ez.sum(axis=1, keepdims=True)
        gt_cls = gt_classes[b].astype(np.int64)
        class_cost = -prob[:, gt_cls]                       # [N, M]
        bce_cost = (sp[:, None] - A) / HW
        dice_cost = 1.0 - (2.0 * B + 1.0) / (ps[:, None] + ys[None, :] + 1.0)
        cost = CLS_W * class_cost + MASK_W * bce_cost + DICE_W * dice_cost

        pi, gi = _hungarian(cost)

        # classification loss (weighted-mean CE, torch semantics)
        logp = z - np.log(ez.sum(axis=1, keepdims=True))
        target = np.full(NQ, NO_OBJ, dtype=np.int64)
        target[pi] = gt_cls[gi]
        nll = -logp[np.arange(NQ), target]
        wts = np.where(target == NO_OBJ, NO_OBJ_W, 1.0)
        cls_loss = (wts * nll).sum() / wts.sum()

        # matched-pair mask bce + dice
        K = pi.shape[0]
        bce = (sp[pi] - A[pi, gi]).sum() / (K * HW)
        dice = (1.0 - (2.0 * B[pi, gi] + 1.0) / (ps[pi] + ys[gi] + 1.0)).mean()

        tc += cls_loss
        tm += bce
        td += dice

    tc, tm, td = tc / BS, tm / BS, td / BS
    total = CLS_W * tc + MASK_W * tm + DICE_W * td
    return np.array([tc, tm, td, total], dtype=np.float32)


def kernel(class_logits, mask_logits, gt_classes, gt_masks):
    class_logits = np.asarray(class_logits)
    mask_logits = np.asarray(mask_logits)
    gt_classes = np.asarray(gt_classes)
    gt_masks = np.asarray(gt_masks)

    in_maps = _prepare_in_maps(mask_logits, gt_masks)
    res = _run_device(in_maps)
    core_outs = [r["out_res"] for r in res.results]
    return _finish_on_host(core_outs, class_logits, gt_classes, gt_masks)


# revision 5
# speedup vs baseline: 2149.5670x; 2149.5670x over previous
"""EoMT criterion (Mask2Former-style loss) on 8 Trainium2 NeuronCores.

Math reduction: for each image with mask logits x [N=100, HW] and binary gt
masks y [M=20, HW], every term of the loss only needs
    A  = x @ y.T                  (since log p - log(1-p) = x)
    B  = sigmoid(x) @ y.T
    sp = sum_px softplus(x)  per row      (= -rowsum log(1-p))
    ps = sum_px sigmoid(x)   per row
    ys = sum_px y            per row (host, exact ints)
because
    bce_cost[n,m]  = (sp[n] - A[n,m]) / HW
    dice_cost[n,m] = 1 - (2 B[n,m] + 1) / (ps[n] + ys[m] + 1)
    matched-pair bce  = sum_k (sp[pi_k] - A[pi_k, gi_k]) / (K*HW)
    matched-pair dice from B/ps/ys at the matched indices.
The device reduces 250MB of inputs to one [21, 300] f32 tile per core; the
Hungarian assignment and the tiny class-logit terms run on host.

Sharding: 8 cores = 2 images x 4 HW-quarters. Host pre-transposes x to
pixel-major [HW, N] bf16 so the pixel (contraction) axis lands on SBUF
partitions, and appends a ones column to y ([HW, 21]) so row 20 of each
matmul output carries the per-row pixel sums (ps, sp).
"""

import numpy as np
import ml_dtypes

BF16 = ml_dtypes.bfloat16

N_CORES = 8
BS = 2
NQ = 100          # number of mask queries
NM = 20           # number of gt masks
NC1 = 7           # classes + no-object
H = W = 512
HW = H * W                      # 262144
PX_PER_CORE = HW // 4           # 65536
N_CHUNKS = 8
PX_PER_CHUNK = PX_PER_CORE // N_CHUNKS   # 8192
F = PX_PER_CHUNK // 128                  # 64 pixel-groups per chunk
NGROUP = PX_PER_CORE // 128              # 512 groups per core

CLS_W, MASK_W, DICE_W, NO_OBJ_W = 2.0, 5.0, 5.0, 0.1
NO_OBJ = 6

_NC_CACHE = {}


def _build_device_kernel(reps=1):
    """One SPMD program per core: inputs xt [65536,100] bf16 (pixel-major
    mask logits), yt [65536,21] bf16 (gt masks + ones col); output
    [21, 300] f32 = [A.T | Q.T | T.T] blocks where A = x @ y1.T,
    Q = sigmoid(-x) @ y1.T, T = ln(sigmoid(-x)) @ y1.T (row 20 = colsums).

    The compiler's activation tables have no softplus, so softplus/sigmoid
    come from q = sigmoid(-x): sigmoid(x) = 1-q, softplus(x) = -ln(q)."""
    import concourse.bacc as bacc
    import concourse.mybir as mybir
    import concourse.tile as tile

    nc = bacc.Bacc("TRN2", target_bir_lowering=False, debug=False,
                   num_devices=N_CORES)
    x_d = nc.dram_tensor("xt", (PX_PER_CORE, NQ), mybir.dt.bfloat16,
                         kind="ExternalInput")
    y_d = nc.dram_tensor("yt", (PX_PER_CORE, NM + 1), mybir.dt.bfloat16,
                         kind="ExternalInput")
    out_d = nc.dram_tensor("out_res", (NM + 1, 3 * NQ), mybir.dt.float32,
                           kind="ExternalOutput")

    AF = mybir.ActivationFunctionType
    bf16 = mybir.dt.bfloat16
    f32 = mybir.dt.float32

    with tile.TileContext(nc) as tc:
        with (
            tc.tile_pool(name="xpool", bufs=3) as xpool,
            tc.tile_pool(name="qpool", bufs=1) as qpool,
            tc.tile_pool(name="ypool", bufs=1) as ypool,
            tc.tile_pool(name="tpool", bufs=2) as tpool,
            tc.tile_pool(name="opool", bufs=1) as opool,
            tc.tile_pool(name="pspool", bufs=1, space="PSUM") as pspool,
        ):
            y_tile = ypool.tile([128, N_CHUNKS, F, NM + 1], bf16)
            nc.sync.dma_start(
                y_tile[:],
                y_d.ap().rearrange("(c p f) j -> p c f j",
                                   c=N_CHUNKS, p=128, f=F))

            psA = pspool.tile([NM + 1, NQ], f32)
            psQ = pspool.tile([NM + 1, NQ], f32)
            psT = pspool.tile([NM + 1, NQ], f32)

            x_view = x_d.ap().rearrange("(c p f) j -> c p f j",
                                        c=N_CHUNKS, p=128, f=F)
            q_tiles = [qpool.tile([128, F, NQ], bf16, name=f"q_{c}")
                       for c in range(N_CHUNKS)]

            def emit_body():
                # phase 1: q = sigmoid(-x); accumulate A (raw x) and Q (q)
                for c in range(N_CHUNKS):
                    x_t = xpool.tile([128, F, NQ], bf16, name="x_t", tag="x")
                    nc.sync.dma_start(x_t[:], x_view[c])
                    q_t = q_tiles[c]
                    nc.scalar.activation(q_t[:], x_t[:], AF.Sigmoid,
                                         scale=-1.0)
                    for f in range(F):
                        g = c * F + f
                        st = g == 0
                        sp = g == NGROUP - 1
                        nc.tensor.matmul(psA[:], y_tile[:, c, f, :],
                                         x_t[:, f, :], start=st, stop=sp)
                        nc.tensor.matmul(psQ[:], y_tile[:, c, f, :],
                                         q_t[:, f, :], start=st, stop=sp)
                # phase 2: t = ln(q) = -softplus(x); accumulate T
                for c in range(N_CHUNKS):
                    q_t = q_tiles[c]
                    t_t = tpool.tile([128, F, NQ], bf16, name="t_t", tag="t")
                    nc.scalar.activation(t_t[:], q_t[:], AF.Ln)
                    for f in range(F):
                        g = c * F + f
                        st = g == 0
                        sp = g == NGROUP - 1
                        nc.tensor.matmul(psT[:], y_tile[:, c, f, :],
                                         t_t[:, f, :], start=st, stop=sp)

            if reps == 1:
                emit_body()
            else:
                # timing mode: repeat the full body (DMA + ACT + PE) inside
                # the NEFF; every iteration recomputes from scratch (start=
                # True clears PSUM), so the final output is still correct.
                with tc.For_i(0, reps, 1):
                    emit_body()

            out_sb = opool.tile([NM + 1, 3 * NQ], f32)
            nc.vector.tensor_copy(out_sb[:, 0:NQ], psA[:])
            nc.vector.tensor_copy(out_sb[:, NQ:2 * NQ], psQ[:])
            nc.vector.tensor_copy(out_sb[:, 2 * NQ:3 * NQ], psT[:])
            nc.sync.dma_start(out_d.ap(), out_sb[:])

    nc.compile()
    return nc


def _get_nc(reps=1):
    if reps not in _NC_CACHE:
        _NC_CACHE[reps] = _build_device_kernel(reps)
    return _NC_CACHE[reps]


def _prepare_in_maps(mask_logits, gt_masks):
    """Host-side marshalling: transpose to pixel-major, cast bf16, shard."""
    m2 = mask_logits.reshape(BS, NQ, HW)
    g2 = gt_masks.reshape(BS, NM, HW)
    in_maps = []
    for b in range(BS):
        for q in range(4):
            sl = slice(q * PX_PER_CORE, (q + 1) * PX_PER_CORE)
            xt = np.ascontiguousarray(m2[b, :, sl].T).astype(BF16)
            yt = np.empty((PX_PER_CORE, NM + 1), dtype=BF16)
            yt[:, :NM] = g2[b, :, sl].T
            yt[:, NM] = BF16(1.0)
            in_maps.append({"xt": xt, "yt": yt})
    return in_maps


def _run_device(in_maps, reps=1, trace=False):
    from concourse import bass_utils
    nc = _get_nc(reps)
    res = bass_utils.run_bass_kernel_spmd(
        nc, in_maps, core_ids=list(range(N_CORES)), trace=trace)
    return res


def _hungarian(cost):
    """Jonker-Volgenant shortest augmenting path; equivalent to scipy's
    linear_sum_assignment. cost [n, m] -> (row_ind, col_ind) sorted by row."""
    cost = np.asarray(cost, dtype=np.float64)
    transposed = cost.shape[0] > cost.shape[1]
    if transposed:
        cost = cost.T
    n, m = cost.shape
    INF = 1e18
    u = np.zeros(n + 1)
    v = np.zeros(m + 1)
    p = np.zeros(m + 1, dtype=np.int64)
    way = np.zeros(m + 1, dtype=np.int64)
    for i in range(1, n + 1):
        p[0] = i
        j0 = 0
        minv = np.full(m + 1, INF)
        used = np.zeros(m + 1, dtype=bool)
        while True:
            used[j0] = True
            i0 = p[j0]
            cand = cost[i0 - 1] - u[i0] - v[1:]
            upd = (~used[1:]) & (cand < minv[1:])
            minv[1:] = np.where(upd, cand, minv[1:])
            way[1:] = np.where(upd, j0, way[1:])
            masked = np.where(used[1:], INF, minv[1:])
            j1 = int(np.argmin(masked)) + 1
            delta = masked[j1 - 1]
            u[p[used]] += delta
            v[used] -= delta
            minv[1:][~used[1:]] -= delta
            j0 = j1
            if p[j0] == 0:
                break
        while j0:
            j1 = way[j0]
            p[j0] = p[j1]
            j0 = j1
    rows, cols = [], []
    for j in range(1, m + 1):
        if p[j] != 0:
            rows.append(p[j] - 1)
            cols.append(j - 1)
    rows = np.asarray(rows, dtype=np.int64)
    cols = np.asarray(cols, dtype=np.int64)
    if transposed:
        rows, cols = cols, rows
    order = np.argsort(rows)
    return rows[order], cols[order]


def _finish_on_host(core_outs, class_logits, gt_classes, gt_masks):
    """Combine per-core [21, 300] partials, assemble costs, match, and
    compute the four loss scalars."""
    g2 = gt_masks.reshape(BS, NM, HW)
    cls64 = class_logits.astype(np.float64)

    tc = tm = td = 0.0
    for b in range(BS):
        tot = np.zeros((NM + 1, 3 * NQ), dtype=np.float64)
        for q in range(4):
            tot += core_outs[4 * b + q].astype(np.float64)
        ys = g2[b].sum(axis=1).astype(np.float64)   # [M]
        A = tot[:NM, 0:NQ].T                 # [N, M] = x @ y.T
        Q = tot[:NM, NQ:2 * NQ].T            # [N, M] = sigmoid(-x) @ y.T
        qs = tot[NM, NQ:2 * NQ]              # [N]    = rowsum sigmoid(-x)
        B = ys[None, :] - Q                  # [N, M] = sigmoid(x) @ y.T
        ps = HW - qs                         # [N]    = rowsum sigmoid(x)
        sp = -tot[NM, 2 * NQ:3 * NQ]         # [N]    = rowsum softplus(x)

        # cost matrix
        cl = cls64[b]                        # [N, 7]
        z = cl - cl.max(axis=1, keepdims=True)
        ez = np.exp(z)
        prob = ez / ez.sum(axis=1, keepdims=True)
        gt_cls = gt_classes[b].astype(np.int64)
        class_cost = -prob[:, gt_cls]                       # [N, M]
        bce_cost = (sp[:, None] - A) / HW
        dice_cost = 1.0 - (2.0 * B + 1.0) / (ps[:, None] + ys[None, :] + 1.0)
        cost = CLS_W * class_cost + MASK_W * bce_cost + DICE_W * dice_cost

        pi, gi = _hungarian(cost)

        # classification loss (weighted-mean CE, torch semantics)
        logp = z - np.log(ez.sum(axis=1, keepdims=True))
        target = np.full(NQ, NO_OBJ, dtype=np.int64)
        target[pi] = gt_cls[gi]
        nll = -logp[np.arange(NQ), target]
        wts = np.where(target == NO_OBJ, NO_OBJ_W, 1.0)
        cls_loss = (wts * nll).sum() / wts.sum()

        # matched-pair mask bce + dice
        K = pi.shape[0]
        bce = (sp[pi] - A[pi, gi]).sum() / (K * HW)
        dice = (1.0 - (2.0 * B[pi, gi] + 1.0) / (ps[pi] + ys[gi] + 1.0)).mean()

        tc += cls_loss
        tm += bce
        td += dice

    tc, tm, td = tc / BS, tm / BS, td / BS
    total = CLS_W * tc + MASK_W * tm + DICE_W * td
    return np.array([tc, tm, td, total], dtype=np.float32)


def kernel(class_logits, mask_logits, gt_classes, gt_masks):
    class_logits = np.asarray(class_logits)
    mask_logits = np.asarray(mask_logits)
    gt_classes = np.asarray(gt_classes)
    gt_masks = np.asarray(gt_masks)

    in_maps = _prepare_in_maps(mask_logits, gt_masks)
    res = _run_device(in_maps)
    core_outs = [r["out_res"] for r in res.results]
    return _finish_on_host(core_outs, class_logits, gt_classes, gt_masks)


# revision 12
# speedup vs baseline: 2583.0517x; 1.2017x over previous
"""EoMT criterion (Mask2Former-style loss) on 8 Trainium2 NeuronCores.

Math reduction: for each image with mask logits x [N=100, HW] and binary gt
masks y [M=20, HW], every term of the loss only needs
    A  = x @ y.T                  (since log p - log(1-p) = x)
    B  = sigmoid(x) @ y.T
    sp = sum_px softplus(x)  per row      (= -rowsum log(1-p))
    ps = sum_px sigmoid(x)   per row
    ys = sum_px y            per row (host, exact ints)
because
    bce_cost[n,m]  = (sp[n] - A[n,m]) / HW
    dice_cost[n,m] = 1 - (2 B[n,m] + 1) / (ps[n] + ys[m] + 1)
    matched-pair bce  = sum_k (sp[pi_k] - A[pi_k, gi_k]) / (K*HW)
    matched-pair dice from B/ps/ys at the matched indices.
The device reduces 250MB of inputs to one [21, 300] f32 tile per core; the
Hungarian assignment and the tiny class-logit terms run on host.

Sharding: 8 cores = 2 images x 4 HW-quarters. Host pre-transposes x to
pixel-major [HW, N] bf16 so the pixel (contraction) axis lands on SBUF
partitions, and appends a ones column to y ([HW, 21]) so row 20 of each
matmul output carries the per-row pixel sums (ps, sp).
"""

import numpy as np
import ml_dtypes

BF16 = ml_dtypes.bfloat16

N_CORES = 8
BS = 2
NQ = 100          # number of mask queries
NM = 20           # number of gt masks
NC1 = 7           # classes + no-object
H = W = 512
HW = H * W                      # 262144
PX_PER_CORE = HW // 4           # 65536
N_CHUNKS = 8
PX_PER_CHUNK = PX_PER_CORE // N_CHUNKS   # 8192
F = PX_PER_CHUNK // 128                  # 64 pixel-groups per chunk
NGROUP = PX_PER_CORE // 128              # 512 groups per core

CLS_W, MASK_W, DICE_W, NO_OBJ_W = 2.0, 5.0, 5.0, 0.1
NO_OBJ = 6

_NC_CACHE = {}


def _build_device_kernel(reps=1, variant="full"):
    """One SPMD program per core: inputs xt [65536,100] bf16 (pixel-major
    mask logits), yt [65536,21] bf16 (gt masks + ones col); output
    [21, 300] f32 = [A.T | Q.T | T.T] blocks where A = x @ y1.T,
    Q = sigmoid(-x) @ y1.T, T = ln(sigmoid(-x)) @ y1.T (row 20 = colsums).

    The compiler's activation tables have no softplus, so softplus/sigmoid
    come from q = sigmoid(-x): sigmoid(x) = 1-q, softplus(x) = -ln(q)."""
    import concourse.bacc as bacc
    import concourse.mybir as mybir
    import concourse.tile as tile

    nc = bacc.Bacc("TRN2", target_bir_lowering=False, debug=False,
                   num_devices=N_CORES)
    x_d = nc.dram_tensor("xt", (PX_PER_CORE, NQ), mybir.dt.bfloat16,
                         kind="ExternalInput")
    y_d = nc.dram_tensor("yt", (PX_PER_CORE, NM + 1), mybir.dt.bfloat16,
                         kind="ExternalInput")
    out_d = nc.dram_tensor("out_res", (NM + 1, 3 * NQ), mybir.dt.float32,
                           kind="ExternalOutput")

    AF = mybir.ActivationFunctionType
    bf16 = mybir.dt.bfloat16
    f32 = mybir.dt.float32

    with tile.TileContext(nc) as tc:
        with (
            tc.tile_pool(name="xpool", bufs=3) as xpool,
            tc.tile_pool(name="qpool", bufs=1) as qpool,
            tc.tile_pool(name="ypool", bufs=1) as ypool,
            tc.tile_pool(name="tpool", bufs=2) as tpool,
            tc.tile_pool(name="opool", bufs=1) as opool,
            tc.tile_pool(name="pspool", bufs=1, space="PSUM") as pspool,
        ):
            y_tile = ypool.tile([128, N_CHUNKS, F, NM + 1], bf16)
            y_view = y_d.ap().rearrange("(c p f) j -> c p f j",
                                        c=N_CHUNKS, p=128, f=F)

            psA = pspool.tile([NM + 1, NQ], f32)
            psQ = pspool.tile([NM + 1, NQ], f32)
            psT = pspool.tile([NM + 1, NQ], f32)

            x_view = x_d.ap().rearrange("(c p f) j -> c p f j",
                                        c=N_CHUNKS, p=128, f=F)
            q_tiles = [qpool.tile([128, F, NQ], bf16, name=f"q_{c}")
                       for c in range(N_CHUNKS)]

            def emit_body():
                do_mm = variant in ("full", "noact")
                do_act = variant in ("full", "nomm")
                # phase 1: q = sigmoid(-x); accumulate A (raw x) and Q (q).
                # Chunk 0 is emitted in slivers so the ACT engine starts as
                # soon as the first slice of x lands.
                for c in range(N_CHUNKS):
                    x_t = xpool.tile([128, F, NQ], bf16, name="x_t", tag="x")
                    q_t = q_tiles[c]
                    splits = 4 if c == 0 else 1
                    sub = F // splits
                    for s in range(splits):
                        sl = slice(s * sub, (s + 1) * sub)
                        nc.sync.dma_start(x_t[:, sl, :], x_view[c][:, sl, :])
                        nc.sync.dma_start(y_tile[:, c, sl, :],
                                          y_view[c][:, sl, :])
                        if do_act:
                            nc.scalar.activation(q_t[:, sl, :], x_t[:, sl, :],
                                                 AF.Sigmoid, scale=-1.0)
                        if do_mm:
                            src1 = q_t if do_act else x_t
                            for f in range(s * sub, (s + 1) * sub):
                                g = c * F + f
                                st = g == 0
                                sp = g == NGROUP - 1
                                nc.tensor.matmul(psA[:], y_tile[:, c, f, :],
                                                 x_t[:, f, :],
                                                 start=st, stop=sp)
                                nc.tensor.matmul(psQ[:], y_tile[:, c, f, :],
                                                 src1[:, f, :],
                                                 start=st, stop=sp)
                # phase 2: t = ln(q) = -softplus(x); accumulate T.
                # The last chunk is emitted in slivers to shrink the tail.
                for c in range(N_CHUNKS):
                    q_t = q_tiles[c]
                    t_t = tpool.tile([128, F, NQ], bf16, name="t_t", tag="t")
                    splits = 4 if c == N_CHUNKS - 1 else 1
                    sub = F // splits
                    for s in range(splits):
                        sl = slice(s * sub, (s + 1) * sub)
                        if do_act:
                            nc.scalar.activation(t_t[:, sl, :], q_t[:, sl, :],
                                                 AF.Ln)
                        if do_mm:
                            src2 = t_t if do_act else q_t
                            for f in range(s * sub, (s + 1) * sub):
                                g = c * F + f
                                st = g == 0
                                sp = g == NGROUP - 1
                                nc.tensor.matmul(psT[:], y_tile[:, c, f, :],
                                                 src2[:, f, :],
                                                 start=st, stop=sp)

            if reps == 1:
                emit_body()
            else:
                # timing mode: repeat the full body (DMA + ACT + PE) inside
                # the NEFF; every iteration recomputes from scratch (start=
                # True clears PSUM), so the final output is still correct.
                with tc.For_i(0, reps, 1):
                    emit_body()

            out_sb = opool.tile([NM + 1, 3 * NQ], f32)
            if variant in ("full", "noact"):
                nc.vector.tensor_copy(out_sb[:, 0:NQ], psA[:])
                nc.vector.tensor_copy(out_sb[:, NQ:2 * NQ], psQ[:])
                nc.vector.tensor_copy(out_sb[:, 2 * NQ:3 * NQ], psT[:])
            else:
                nc.gpsimd.memset(out_sb[:], 0.0)
            nc.sync.dma_start(out_d.ap(), out_sb[:])

    nc.compile()
    return nc


def _get_nc(reps=1, variant="full"):
    key = (reps, variant)
    if key not in _NC_CACHE:
        _NC_CACHE[key] = _build_device_kernel(reps, variant)
    return _NC_CACHE[key]


def _prepare_in_maps(mask_logits, gt_masks):
    """Host-side marshalling: transpose to pixel-major, cast bf16, shard."""
    m2 = mask_logits.reshape(BS, NQ, HW)
    g2 = gt_masks.reshape(BS, NM, HW)
    in_maps = []
    for b in range(BS):
        for q in range(4):
            sl = slice(q * PX_PER_CORE, (q + 1) * PX_PER_CORE)
            xt = np.ascontiguousarray(m2[b, :, sl].T).astype(BF16)
            yt = np.empty((PX_PER_CORE, NM + 1), dtype=BF16)
            yt[:, :NM] = g2[b, :, sl].T
            yt[:, NM] = BF16(1.0)
            in_maps.append({"xt": xt, "yt": yt})
    return in_maps


def _run_device(in_maps, reps=1, variant="full", trace=False):
    from concourse import bass_utils
    nc = _get_nc(reps, variant)
    res = bass_utils.run_bass_kernel_spmd(
        nc, in_maps, core_ids=list(range(N_CORES)), trace=trace)
    return res


def _hungarian(cost):
    """Jonker-Volgenant shortest augmenting path; equivalent to scipy's
    linear_sum_assignment. cost [n, m] -> (row_ind, col_ind) sorted by row."""
    cost = np.asarray(cost, dtype=np.float64)
    transposed = cost.shape[0] > cost.shape[1]
    if transposed:
        cost = cost.T
    n, m = cost.shape
    INF = 1e18
    u = np.zeros(n + 1)
    v = np.zeros(m + 1)
    p = np.zeros(m + 1, dtype=np.int64)
    way = np.zeros(m + 1, dtype=np.int64)
    for i in range(1, n + 1):
        p[0] = i
        j0 = 0
        minv = np.full(m + 1, INF)
        used = np.zeros(m + 1, dtype=bool)
        while True:
            used[j0] = True
            i0 = p[j0]
            cand = cost[i0 - 1] - u[i0] - v[1:]
            upd = (~used[1:]) & (cand < minv[1:])
            minv[1:] = np.where(upd, cand, minv[1:])
            way[1:] = np.where(upd, j0, way[1:])
            masked = np.where(used[1:], INF, minv[1:])
            j1 = int(np.argmin(masked)) + 1
            delta = masked[j1 - 1]
            u[p[used]] += delta
            v[used] -= delta
            minv[1:][~used[1:]] -= delta
            j0 = j1
            if p[j0] == 0:
                break
        while j0:
            j1 = way[j0]
            p[j0] = p[j1]
            j0 = j1
    rows, cols = [], []
    for j in range(1, m + 1):
        if p[j] != 0:
            rows.append(p[j] - 1)
            cols.append(j - 1)
    rows = np.asarray(rows, dtype=np.int64)
    cols = np.asarray(cols, dtype=np.int64)
    if transposed:
        rows, cols = cols, rows
    order = np.argsort(rows)
    return rows[order], cols[order]


def _finish_on_host(core_outs, class_logits, gt_classes, gt_masks):
    """Combine per-core [21, 300] partials, assemble costs, match, and
    compute the four loss scalars."""
    g2 = gt_masks.reshape(BS, NM, HW)
    cls64 = class_logits.astype(np.float64)

    tc = tm = td = 0.0
    for b in range(BS):
        tot = np.zeros((NM + 1, 3 * NQ), dtype=np.float64)
        for q in range(4):
            tot += core_outs[4 * b + q].astype(np.float64)
        ys = g2[b].sum(axis=1).astype(np.float64)   # [M]
        A = tot[:NM, 0:NQ].T                 # [N, M] = x @ y.T
        Q = tot[:NM, NQ:2 * NQ].T            # [N, M] = sigmoid(-x) @ y.T
        qs = tot[NM, NQ:2 * NQ]              # [N]    = rowsum sigmoid(-x)
        B = ys[None, :] - Q                  # [N, M] = sigmoid(x) @ y.T
        ps = HW - qs                         # [N]    = rowsum sigmoid(x)
        sp = -tot[NM, 2 * NQ:3 * NQ]         # [N]    = rowsum softplus(x)

        # cost matrix
        cl = cls64[b]                        # [N, 7]
        z = cl - cl.max(axis=1, keepdims=True)
        ez = np.exp(z)
        prob = ez / ez.sum(axis=1, keepdims=True)
        gt_cls = gt_classes[b].astype(np.int64)
        class_cost = -prob[:, gt_cls]                       # [N, M]
        bce_cost = (sp[:, None] - A) / HW
        dice_cost = 1.0 - (2.0 * B + 1.0) / (ps[:, None] + ys[None, :] + 1.0)
        cost = CLS_W * class_cost + MASK_W * bce_cost + DICE_W * dice_cost

        pi, gi = _hungarian(cost)

        # classification loss (weighted-mean CE, torch semantics)
        logp = z - np.log(ez.sum(axis=1, keepdims=True))
        target = np.full(NQ, NO_OBJ, dtype=np.int64)
        target[pi] = gt_cls[gi]
        nll = -logp[np.arange(NQ), target]
        wts = np.where(target == NO_OBJ, NO_OBJ_W, 1.0)
        cls_loss = (wts * nll).sum() / wts.sum()

        # matched-pair mask bce + dice
        K = pi.shape[0]
        bce = (sp[pi] - A[pi, gi]).sum() / (K * HW)
        dice = (1.0 - (2.0 * B[pi, gi] + 1.0) / (ps[pi] + ys[gi] + 1.0)).mean()

        tc += cls_loss
        tm += bce
        td += dice

    tc, tm, td = tc / BS, tm / BS, td / BS
    total = CLS_W * tc + MASK_W * tm + DICE_W * td
    return np.array([tc, tm, td, total], dtype=np.float32)


def kernel(class_logits, mask_logits, gt_classes, gt_masks):
    class_logits = np.asarray(class_logits)
    mask_logits = np.asarray(mask_logits)
    gt_classes = np.asarray(gt_classes)
    gt_masks = np.asarray(gt_masks)

    in_maps = _prepare_in_maps(mask_logits, gt_masks)
    res = _run_device(in_maps)
    core_outs = [r["out_res"] for r in res.results]
    return _finish_on_host(core_outs, class_logits, gt_classes, gt_masks)
